# revision 1
# baseline (speedup 1.0000x reference)
"""Distance-encoded-bias multi-head self-attention on 8 Trainium2 NeuronCores.

Strategy
--------
Shard (batch b in 0..1) x (head-pair in 0..3) -> 8 cores. Each core computes
its 2 heads' full attention for its batch, plus the output-projection partial
for its heads' rows of proj_w; the host sums the 4 partials per batch.

Key algebraic moves (all exact):
 * Tokens are sorted by coordinate on the host (attention is permutation
   equivariant; output rows are inverse-permuted back).
 * cos(w|xi-xj|) = C_i C_j + S_i S_j with C=cos(w x), S=sin(w x), and
   sin(w|xi-xj|) = sign(xi-xj)(S_i C_j - C_i S_j). With sorted coords the
   sign is +/-1 by block position (triangular only on diagonal blocks), so
   the whole Fourier bias becomes rank-2F matmuls -- no N^2 transcendentals.
 * The Gaussian local term E = exp(-d^2/ell^2) is precomputed on the host
   (head-independent: all heads share ell) and added into the score PSUM by
   the tensor engine via a ta_h-scaled identity matmul.
 * Scores are built transposed (keys on partitions, queries free). Softmax
   uses a per-query upper bound C_i instead of a row max (any per-query shift
   cancels in softmax); C_i rides the score matmul as one extra rank-1 row.
   The denominator comes from a ones-column appended to V in the attn@V
   matmul and is divided out after attn@V.
 * Matmul operands are fp32 bitcast to float32r (1 cycle/row vs 4 for fp32);
   PSUM accumulation stays fp32.
"""

import math

import numpy as np

B, N, DIM, H, NF = 2, 1024, 512, 8, 8
HD = DIM // H
SCALE = HD ** -0.5
NCORES = 8
CHUNK = 128
NCHUNKS = N // CHUNK

_PROGRAM_CACHE = {}


def _bf16():
    import ml_dtypes

    return ml_dtypes.bfloat16


def _softplus64(x):
    x = np.asarray(x, np.float64)
    return np.log1p(np.exp(-np.abs(x))) + np.maximum(x, 0.0) + 1e-12


def _split_excess_waits(nc, max_waits=1):
    """CoreV3 walrus allows only one sync-wait command on some instruction
    encodings; move excess waits onto preceding same-engine NoOps."""
    import concourse.mybir as mybir
    import bass_rust

    n_split = 0
    for bb in nc.main_func.blocks:
        new_list = []
        changed = False
        for ins in bb.instructions:
            si = ins.sync_info
            waits = list(si.on_wait) if (si and si.on_wait) else []
            if len(waits) > max_waits:
                changed = True
                extra, keep = waits[:-max_waits], waits[-max_waits:]
                for i in range(0, len(extra), max_waits):
                    chunk = extra[i : i + max_waits]
                    n_split += 1
                    new_list.append(
                        mybir.InstNoOp(
                            name=f"{ins.name}-ws{i}",
                            engine=ins.engine,
                            ins=[],
                            outs=[],
                            sync_info=bass_rust.SyncInfo(
                                on_wait=chunk, on_update=[]
                            ),
                        )
                    )
                si.on_wait = keep
            new_list.append(ins)
        if changed:
            bb.instructions = new_list
    return n_split


def _build_program(biases_zero=True, repeat=1):
    key = ("nc", biases_zero, repeat)
    if key in _PROGRAM_CACHE:
        return _PROGRAM_CACHE[key]

    import concourse.bass as bass
    import concourse.mybir as mybir
    import concourse.tile as tile

    f32 = mybir.dt.float32
    f32r = mybir.dt.float32r
    Alu = mybir.AluOpType
    Act = mybir.ActivationFunctionType

    nc = bass.Bass(trn_type="TRN2")

    # ---- per-core DRAM I/O ------------------------------------------------
    xt_d = nc.dram_tensor("xt", [DIM, N], f32r, kind="ExternalInput")
    identx_d = nc.dram_tensor("identx", [128, 384], f32r, kind="ExternalInput")
    wqk_d = nc.dram_tensor("wqk", [DIM, 256], f32r, kind="ExternalInput")
    wv_d = nc.dram_tensor("wv", [DIM, 128], f32r, kind="ExternalInput")
    wproj_d = nc.dram_tensor("wproj", [128, DIM], f32r, kind="ExternalInput")
    qb_d = nc.dram_tensor("qb", [2, HD, 1], f32, kind="ExternalInput")
    kb_d = nc.dram_tensor("kb", [2, HD, 1], f32, kind="ExternalInput")
    # 33 rows: C,S feats (16) | ones or -Ci (1) | sin-side combos (16)
    kext_d = nc.dram_tensor("kext", [2, 33, N], f32r, kind="ExternalInput")
    qextp_d = nc.dram_tensor("qextp", [2, 33, N], f32r, kind="ExternalInput")
    qextm_d = nc.dram_tensor("qextm", [2, 33, N], f32r, kind="ExternalInput")
    # column-packed [qc | qcn | feat], each N wide
    qcs_d = nc.dram_tensor("qcs", [2, 16, 3 * N], f32r, kind="ExternalInput")
    erl_d = nc.dram_tensor("erl", [3, N], f32r, kind="ExternalInput")
    errr_d = nc.dram_tensor("errr", [3, 8 * N], f32r, kind="ExternalInput")
    dwin_d = nc.dram_tensor("dwin", [128, 16 * 128], f32r, kind="ExternalInput")
    ones64_d = nc.dram_tensor("ones64", [1, 64], f32, kind="ExternalInput")
    onescol_d = nc.dram_tensor("onescol", [128, 1], f32r, kind="ExternalInput")
    yt_d = nc.dram_tensor("yt", [DIM, N], f32, kind="ExternalOutput")

    with tile.TileContext(nc) as tc:
      for _rep in range(repeat):
        with (
            tc.tile_pool(name="persist", bufs=1) as pers,
            tc.tile_pool(name="work", bufs=3) as work,
            tc.tile_pool(name="dmw", bufs=2) as dmw,
            tc.tile_pool(name="yg", bufs=2) as ygp,
        ):
            # ---- persistent SBUF tiles + input DMA, issued in the order the
            # prolog consumes them (xs -> wqk -> score-side smalls -> wv -> E)
            def pt(shape, tag, dt=f32):
                return pers.tile(shape, dt, tag=tag, name=tag)

            erl_t = pt([3, N], "erl", f32r)
            nc.sync.dma_start(erl_t[:], erl_d[:])
            errr_t = pt([3, 8 * N], "errr", f32r)
            nc.sync.dma_start(errr_t[:], errr_d[:])
            e_t = [pt([128, N], f"e{k}", f32r) for k in range(NCHUNKS)]
            xT_t = []
            for c in range(4):
                s = pt([128, N], f"xT{c}", f32r)
                nc.sync.dma_start(s[:], xt_d[c * 128 : (c + 1) * 128, :])
                xT_t.append(s)
            wqk_t = []
            for kc in range(4):
                s = pt([128, 256], f"wqk{kc}", f32r)
                nc.sync.dma_start(s[:], wqk_d[kc * 128 : (kc + 1) * 128, :])
                wqk_t.append(s)

            kf_t = [pt([97, N], f"kf{h}", f32r) for h in range(2)]
            qap_t = [pt([97, N], f"qap{h}", f32r) for h in range(2)]
            qam_t = [pt([97, N], f"qam{h}", f32r) for h in range(2)]
            qcs_t, qc_t, qcn_t, feat_t = [], [], [], []
            for h in range(2):
                nc.sync.dma_start(kf_t[h][64:97, :], kext_d[h])
                nc.sync.dma_start(qap_t[h][64:97, :], qextp_d[h])
                nc.sync.dma_start(qam_t[h][64:97, :], qextm_d[h])
                s = pt([16, 3 * N], f"qcs{h}", f32r)
                nc.sync.dma_start(s[:], qcs_d[h])
                qcs_t.append(s)
                qc_t.append(s[:, 0:N])
                qcn_t.append(s[:, N : 2 * N])
                feat_t.append(s[:, 2 * N : 3 * N])
            qb_t, kb_t = [], []
            if not biases_zero:
                for h in range(2):
                    s = pt([HD, 1], f"qb{h}")
                    nc.sync.dma_start(s[:], qb_d[h])
                    qb_t.append(s)
                    s = pt([HD, 1], f"kb{h}")
                    nc.sync.dma_start(s[:], kb_d[h])
                    kb_t.append(s)
            bf16 = mybir.dt.bfloat16
            wv_t = []
            for kc in range(4):
                s = pt([128, 128], f"wv{kc}", f32r)
                nc.sync.dma_start(s[:], wv_d[kc * 128 : (kc + 1) * 128, :])
                wv_t.append(s)
            identx_t = pt([128, 384], "identx", f32r)
            nc.sync.dma_start(identx_t[:], identx_d[:])
            identa_t = [identx_t[:, 0:128], identx_t[:, 128:256]]
            identr_t = identx_t[:, 256:384]
            dwin_t = pt([128, 16 * 128], "dwin", f32r)
            nc.sync.dma_start(dwin_t[:], dwin_d[:])
            wproj_t = pt([128, DIM], "wproj", f32r)
            nc.sync.dma_start(wproj_t[:], wproj_d[:])
            ones64_t = pt([1, 64], "ones64")
            nc.sync.dma_start(ones64_t[:], ones64_d[:])

            onescol_t = pt([128, 1], "onescol", f32r)
            nc.sync.dma_start(onescol_t[:], onescol_d[:])
            vo_t = [[pt([128, 65], f"vo{h}_{t}", f32r) for t in range(8)] for h in range(2)]
            os_t = pt([128, N], "os", f32r)

            # ---- prolog: x^T, qk^T, v ------------------------------------
            with (
                tc.tile_pool(name="ppro", bufs=2, space="PSUM") as ppro,
                tc.tile_pool(name="pv", bufs=2, space="PSUM") as pvp,
                tc.tile_pool(name="pe2", bufs=2, space="PSUM") as pep,
            ):
                # E(k) = exp(-d^2/ell^2): d^2 is rank-3 in the sorted coords,
                # so PE+ACT build all of E during the DMA-bound window.
                for k in range(NCHUNKS):
                    j0 = k * 128
                    for nh in range(2):
                        pe2 = pep.tile([128, 512], f32, tag="pe2")
                        nc.tensor.matmul(
                            pe2[:],
                            lhsT=erl_t[:, j0 : j0 + 128],
                            rhs=errr_t[:, k * N + nh * 512 : k * N + (nh + 1) * 512],
                            start=True, stop=True, skip_group_check=True,
                        )
                        nc.scalar.activation(
                            e_t[k][:, nh * 512 : (nh + 1) * 512],
                            pe2[:], Act.Exp,
                        )
                for h in (0,):
                    p = ppro.tile([128, N], f32, tag="ppro")
                    for kc in range(4):
                        for nh in range(2):
                            nc.tensor.matmul(
                                p[:, nh * 512 : (nh + 1) * 512],
                                lhsT=wqk_t[kc][:, h * 128 : (h + 1) * 128],
                                rhs=xT_t[kc][:, nh * 512 : (nh + 1) * 512],
                                start=(kc == 0),
                                stop=(kc == 3),
                            )
                    if biases_zero:
                        for nh in range(2):
                            cs_ = slice(nh * 512, (nh + 1) * 512)
                            nc.scalar.mul(
                                qap_t[h][0:64, cs_], p[0:64, cs_], SCALE
                            )
                            nc.vector.tensor_scalar_mul(
                                qam_t[h][0:64, cs_], p[0:64, cs_], SCALE
                            )
                            nc.vector.tensor_copy(
                                kf_t[h][0:64, cs_], p[64:128, cs_]
                            )
                    else:
                        nc.vector.tensor_scalar(
                            qap_t[h][0:64, :], p[0:64, :],
                            scalar1=qb_t[h][:], scalar2=SCALE,
                            op0=Alu.add, op1=Alu.mult,
                        )
                        nc.vector.tensor_scalar(
                            qam_t[h][0:64, :], p[0:64, :],
                            scalar1=qb_t[h][:], scalar2=SCALE,
                            op0=Alu.add, op1=Alu.mult,
                        )
                        nc.vector.tensor_scalar(
                            kf_t[h][0:64, :], p[64:128, :],
                            scalar1=kb_t[h][:], scalar2=None, op0=Alu.add,
                        )

                for t in range(8):
                    p = pvp.tile([128, 128], f32, tag="pv")
                    for kc in range(4):
                        nc.tensor.matmul(
                            p[:],
                            lhsT=xT_t[kc][:, t * 128 : (t + 1) * 128],
                            rhs=wv_t[kc][:],
                            start=(kc == 0),
                            stop=(kc == 3),
                        )
                    for h in range(2):
                        nc.vector.tensor_copy(
                            vo_t[h][t][:, 0:64], p[:, h * 64 : (h + 1) * 64]
                        )
                        nc.vector.tensor_copy(vo_t[h][t][:, 64:65], onescol_t[:])

                for h in (1,):
                    p = ppro.tile([128, N], f32, tag="ppro")
                    for kc in range(4):
                        for nh in range(2):
                            nc.tensor.matmul(
                                p[:, nh * 512 : (nh + 1) * 512],
                                lhsT=wqk_t[kc][:, h * 128 : (h + 1) * 128],
                                rhs=xT_t[kc][:, nh * 512 : (nh + 1) * 512],
                                start=(kc == 0),
                                stop=(kc == 3),
                            )
                    if biases_zero:
                        for nh in range(2):
                            cs_ = slice(nh * 512, (nh + 1) * 512)
                            nc.scalar.mul(
                                qap_t[h][0:64, cs_], p[0:64, cs_], SCALE
                            )
                            nc.vector.tensor_scalar_mul(
                                qam_t[h][0:64, cs_], p[0:64, cs_], SCALE
                            )
                            nc.vector.tensor_copy(
                                kf_t[h][0:64, cs_], p[64:128, cs_]
                            )
                    else:
                        nc.vector.tensor_scalar(
                            qap_t[h][0:64, :], p[0:64, :],
                            scalar1=qb_t[h][:], scalar2=SCALE,
                            op0=Alu.add, op1=Alu.mult,
                        )
                        nc.vector.tensor_scalar(
                            qam_t[h][0:64, :], p[0:64, :],
                            scalar1=qb_t[h][:], scalar2=SCALE,
                            op0=Alu.add, op1=Alu.mult,
                        )
                        nc.vector.tensor_scalar(
                            kf_t[h][0:64, :], p[64:128, :],
                            scalar1=kb_t[h][:], scalar2=None, op0=Alu.add,
                        )

            # ---- main attention loop (attn@V pipelined one chunk back) ----
            with (
                tc.tile_pool(name="pp", bufs=2, space="PSUM") as ppp,
                tc.tile_pool(name="po", bufs=2, space="PSUM") as pop,
            ):
                def scores_chunk(h, k, p):
                    j0 = k * 128
                    ch = slice(j0, j0 + 128)
                    for half in range(2):
                        h0 = half * 512
                        cols = slice(h0, h0 + 512)
                        if not (h0 <= j0 < h0 + 512):
                            # whole half has one sign: single K=97 matmul
                            src = qam_t[h] if j0 > h0 else qap_t[h]
                            nc.tensor.matmul(
                                p[:, cols], lhsT=kf_t[h][:, ch],
                                rhs=src[:, cols],
                                start=True, stop=False, skip_group_check=True,
                            )
                        else:
                            nc.tensor.matmul(
                                p[:, cols], lhsT=kf_t[h][0:81, ch],
                                rhs=qap_t[h][0:81, cols],
                                start=True, stop=False, skip_group_check=True,
                            )
                            for c0, c1, neg in (
                                (h0, j0, True),
                                (j0 + 128, h0 + 512, False),
                            ):
                                if c1 <= c0:
                                    continue
                                src = qcn_t[h] if neg else qc_t[h]
                                nc.tensor.matmul(
                                    p[:, c0:c1], lhsT=feat_t[h][:, ch],
                                    rhs=src[:, c0:c1],
                                    start=False, stop=False,
                                    skip_group_check=True,
                                )
                            # host-computed signed sin-part for the diagonal
                            # 128x128 window, added via identity matmul
                            nc.tensor.matmul(
                                p[:, ch], lhsT=identr_t,
                                rhs=dwin_t[
                                    :, (h * 8 + k) * 128 : (h * 8 + k + 1) * 128
                                ],
                                start=False, stop=False,
                                skip_group_check=True,
                            )
                        # local Gaussian term via ta_h-scaled identity
                        nc.tensor.matmul(
                            p[:, cols], lhsT=identa_t[h],
                            rhs=e_t[k][:, cols],
                            start=False, stop=True, skip_group_check=True,
                        )
                    xb = work.tile([128, N], f32r, tag="xb")
                    nc.scalar.activation(xb[:], p[:], Act.Exp)
                    return xb

                def attnv_chunk(h, k, xb, o):
                    for nh in range(2):
                        nc.tensor.matmul(
                            o[0:65, nh * 512 : (nh + 1) * 512],
                            lhsT=vo_t[h][k][:],
                            rhs=xb[:, nh * 512 : (nh + 1) * 512],
                            start=(k == 0),
                            stop=(k == NCHUNKS - 1),
                            skip_group_check=True,
                        )

                for h in range(2):
                    o = pop.tile([128, N], f32, tag="po")
                    pending = None  # (k, xb) awaiting attn@V
                    for k in range(NCHUNKS):
                        p = ppp.tile([128, N], f32, tag="pp")
                        xb = scores_chunk(h, k, p)
                        if pending is not None:
                            attnv_chunk(h, pending[0], pending[1], o)
                        pending = (k, xb)
                    attnv_chunk(h, pending[0], pending[1], o)

                    # normalize: OS[h] = O[0:64] * (1/denominator)
                    rcp = dmw.tile([1, N], f32, tag="rr")
                    nc.vector.reciprocal(rcp[:], o[64:65, :])
                    rb = ppp.tile([64, N], f32, tag="pp")
                    for nh in range(2):
                        nc.tensor.matmul(
                            rb[:, nh * 512 : (nh + 1) * 512],
                            lhsT=ones64_t[:],
                            rhs=rcp[:, nh * 512 : (nh + 1) * 512],
                            start=True, stop=True, skip_group_check=True,
                        )
                    nc.scalar.copy(os_t[h * 64 : (h + 1) * 64, :], o[0:64, :])
                    nc.vector.tensor_tensor(
                        os_t[h * 64 : (h + 1) * 64, :],
                        os_t[h * 64 : (h + 1) * 64, :],
                        rb[:],
                        op=Alu.mult,
                    )

                # ---- projection partial ----------------------------------
                for g in range(4):
                    p = ppp.tile([128, N], f32, tag="pp")
                    for nh in range(2):
                        nc.tensor.matmul(
                            p[:, nh * 512 : (nh + 1) * 512],
                            lhsT=wproj_t[:, g * 128 : (g + 1) * 128],
                            rhs=os_t[:, nh * 512 : (nh + 1) * 512],
                            start=True, stop=True, skip_group_check=True,
                        )
                    yg = ygp.tile([128, N], f32, tag="yg")
                    if g % 2 == 0:
                        nc.scalar.copy(yg[:], p[:])
                    else:
                        nc.vector.tensor_copy(yg[:], p[:])
                    nc.sync.dma_start(yt_d[g * 128 : (g + 1) * 128, :], yg[:])

    _split_excess_waits(nc)
    _PROGRAM_CACHE[key] = nc
    return nc


def _prepare_in_maps(
    x_tokens, coords, qkv_w, qkv_b, proj_w, omega_raw, a, c,
    alpha_raw, ell_raw, bias_scale_raw,
):
    """Host-side preprocessing. Returns (in_maps, perms)."""
    x64 = np.asarray(x_tokens, np.float64)
    co64 = np.asarray(coords, np.float64)
    w64 = np.asarray(qkv_w, np.float64)
    wb64 = np.asarray(qkv_b, np.float64)

    alpha = _softplus64(alpha_raw)            # (H,)
    ell = _softplus64(ell_raw)                # (H,)
    om = _softplus64(omega_raw)               # (H, F)
    t = np.tanh(np.asarray(bias_scale_raw, np.float64))  # (H,)
    a2 = t[:, None] * np.asarray(a, np.float64)          # (H, F)
    c2 = t[:, None] * np.asarray(c, np.float64)
    ta = t * alpha                                        # (H,)

    assert np.allclose(ell, ell[0]), "per-head ell not supported"

    ident = np.eye(128, dtype=np.float32)
    io, jo = np.meshgrid(np.arange(128), np.arange(128), indexing="ij")
    tri = np.sign(jo - io).astype(np.float64)  # TRI[p, c] = sign(c - p)

    perms, in_maps = [], []
    for b in range(B):
        perm = np.argsort(co64[b], kind="stable")
        perms.append(perm)
        cs = co64[b][perm]                      # sorted coords
        xs = x64[b][perm]                       # (N, DIM)
        l2 = ell[0] ** 2
        # per-key-chunk centering keeps the rank-3 d^2 factors small where
        # E = exp(-d^2/l^2) is non-negligible (f32r has ~12 mantissa bits)
        erl = np.empty((3, N))
        errr = np.empty((NCHUNKS, 3, N))
        for kk in range(NCHUNKS):
            mu = cs[kk * 128 : (kk + 1) * 128].mean()
            u = cs[kk * 128 : (kk + 1) * 128] - mu
            erl[:, kk * 128 : (kk + 1) * 128] = np.stack(
                [u**2, u, np.ones(128)]
            )
            w = cs - mu
            errr[kk] = np.stack([-np.ones(N) / l2, 2 * w / l2, -(w**2) / l2])
        erl = erl.astype(np.float32)
        # (NCHUNKS,3,N) -> (3, NCHUNKS*N) with chunk-k block at cols [k*N,(k+1)*N)
        errr = np.concatenate(list(errr), axis=1).astype(np.float32)

        # all-head q/k in f64 for the per-query shift bound
        qk = xs @ w64[:, : 2 * DIM] + wb64[: 2 * DIM]
        for pair in range(4):
            heads = (2 * pair, 2 * pair + 1)
            wqk_cols, wv_cols = [], []
            qb_rows, kb_rows = [], []
            kext, qextp, qextm, qcs = [], [], [], []
            identa, dwin = [], []
            for h in heads:
                sl_q = slice(h * HD, (h + 1) * HD)
                sl_k = slice(DIM + h * HD, DIM + (h + 1) * HD)
                sl_v = slice(2 * DIM + h * HD, 2 * DIM + (h + 1) * HD)
                wqk_cols.append(np.asarray(qkv_w)[:, sl_q])
                wqk_cols.append(np.asarray(qkv_w)[:, sl_k])
                wv_cols.append(np.asarray(qkv_w)[:, sl_v])
                qb_rows.append(np.asarray(qkv_b)[sl_q])
                kb_rows.append(np.asarray(qkv_b)[sl_k])

                C = np.cos(om[h][:, None] * cs[None, :])   # (F, N)
                S = np.sin(om[h][:, None] * cs[None, :])
                kext.append(np.concatenate([C, S, np.ones((1, N)), C, S], axis=0))
                qn = np.linalg.norm(qk[:, sl_q], axis=1)   # (N,)
                kmax = np.linalg.norm(qk[:, sl_k], axis=1).max()
                bb = abs(ta[h]) + np.abs(a2[h]).sum() + np.abs(c2[h]).sum()
                ci = SCALE * qn * kmax + bb + 1.0          # (N,)
                qa_rows = np.concatenate(
                    [a2[h][:, None] * C, a2[h][:, None] * S, -ci[None, :]],
                    axis=0,
                )  # (17, N)
                qc_rows = np.concatenate(
                    [c2[h][:, None] * S, -c2[h][:, None] * C], axis=0
                )  # (16, N)
                qextp.append(np.concatenate([qa_rows, qc_rows], axis=0))
                qextm.append(np.concatenate([qa_rows, -qc_rows], axis=0))
                featcs_rows = None
                qcs.append(
                    np.concatenate(
                        [qc_rows, -qc_rows, np.concatenate([C, S], axis=0)],
                        axis=1,
                    )
                )
                identa.append(np.eye(128) * ta[h])
                # signed sin-part on each diagonal 128x128 window:
                # dwin[k][j, i] = tri[j, i] * sum_r featCS[r, j0+j] qc[r, j0+i]
                featcs = np.concatenate([C, S], axis=0)      # (16, N)
                wins = []
                for k in range(NCHUNKS):
                    j0 = k * 128
                    blk = featcs[:, j0 : j0 + 128].T @ qc_rows[:, j0 : j0 + 128]
                    wins.append(blk * tri)
                dwin.append(np.stack(wins))

            in_maps.append(
                {
                    "xt": np.ascontiguousarray(xs.T, dtype=np.float32),
                    "identx": np.concatenate(
                        identa + [np.eye(128)], axis=1
                    ).astype(np.float32),
                    "wqk": np.ascontiguousarray(
                        np.concatenate(wqk_cols, axis=1), dtype=np.float32
                    ),
                    "wv": np.ascontiguousarray(
                        np.concatenate(wv_cols, axis=1), dtype=np.float32
                    ),
                    "wproj": np.ascontiguousarray(
                        np.asarray(proj_w)[
                            heads[0] * HD : (heads[1] + 1) * HD, :
                        ],
                        dtype=np.float32,
                    ),
                    "qb": np.stack(qb_rows).astype(np.float32)[:, :, None],
                    "kb": np.stack(kb_rows).astype(np.float32)[:, :, None],
                    "kext": np.stack(kext).astype(np.float32),
                    "qextp": np.stack(qextp).astype(np.float32),
                    "qextm": np.stack(qextm).astype(np.float32),
                    "qcs": np.stack(qcs).astype(np.float32),
                    "erl": erl,
                    "errr": errr,
                    "identr": ident,
                    "dwin": np.concatenate(
                        [w for hw in dwin for w in hw], axis=1
                    ).astype(np.float32),
                    "ones64": np.ones((1, 64), np.float32),
                    "onescol": np.ones((128, 1), np.float32),
                }
            )
    return in_maps, perms


def kernel(
    x_tokens, coords, qkv_w, qkv_b, proj_w, proj_b,
    omega_raw, a, c, alpha_raw, ell_raw, bias_scale_raw,
):
    from concourse.bass_utils import run_bass_kernel_spmd

    biases_zero = not np.any(np.asarray(qkv_b))
    nc = _build_program(biases_zero=biases_zero)
    in_maps, perms = _prepare_in_maps(
        x_tokens, coords, qkv_w, qkv_b, proj_w, omega_raw, a, c,
        alpha_raw, ell_raw, bias_scale_raw,
    )
    res = run_bass_kernel_spmd(nc, in_maps, core_ids=list(range(NCORES)))

    # v-bias contributes a constant row (attention weights sum to 1)
    vb = np.asarray(qkv_b, np.float64)[2 * DIM :]
    const_row = vb @ np.asarray(proj_w, np.float64) + np.asarray(
        proj_b, np.float64
    )

    out = np.empty((B, N, DIM), np.float32)
    for b in range(B):
        acc = np.zeros((N, DIM), np.float64)
        for pair in range(4):
            acc += res.results[4 * b + pair]["yt"].T.astype(np.float64)
        acc += const_row[None, :]
        y = np.empty((N, DIM), np.float64)
        y[perms[b]] = acc
        out[b] = y.astype(np.float32)
    return out



# revision 23
# speedup vs baseline: 70.1544x; 70.1544x over previous
"""Distance-encoded-bias multi-head self-attention on 8 Trainium2 NeuronCores.

Strategy
--------
Shard (batch b in 0..1) x (head-pair in 0..3) -> 8 cores. Each core computes
its 2 heads' full attention for its batch, plus the output-projection partial
for its heads' rows of proj_w; the host sums the 4 partials per batch.

Key algebraic moves (all exact):
 * Tokens are sorted by coordinate on the host (attention is permutation
   equivariant; output rows are inverse-permuted back).
 * cos(w|xi-xj|) = C_i C_j + S_i S_j with C=cos(w x), S=sin(w x), and
   sin(w|xi-xj|) = sign(xi-xj)(S_i C_j - C_i S_j). With sorted coords the
   sign is +/-1 by block position (triangular only on diagonal blocks), so
   the whole Fourier bias becomes rank-2F matmuls -- no N^2 transcendentals.
 * The Gaussian local term E = exp(-d^2/ell^2) decays fast; with sorted
   coords it is only non-negligible within a +/-BAND chunk band around the
   diagonal. E is built on device from a rank-3 expansion of d^2 (PE+ACT),
   stored fp16, and added into the score PSUM via ta_h-scaled fp16 identity
   matmuls over band columns only.
 * Scores are built transposed (keys on partitions, queries free). Softmax
   uses a per-query upper bound C_i instead of a row max (any per-query shift
   cancels in softmax); C_i rides the score matmul as one extra rank-1 row.
   The denominator comes from a ones-column appended to V in the attn@V
   matmul and is divided out after attn@V.
 * V is computed transposed (v^T = wv^T x^T, full-width matmuls) and flipped
   back with PE transposes: avoids the 4x f32r penalty on narrow matmuls.
 * Matmul operands are fp32 bitcast to float32r (1 cycle/row vs 4 for fp32);
   small host-precomputed bias factors (Fourier features, diagonal sin
   windows) are fp16 (1 cycle/row at any width). PSUM accumulation is fp32.
 * DMAs are packed into ~11 transfers (HWDGE dispatch is ~625ns per DMA).
"""

import math

import numpy as np

B, N, DIM, H, NF = 2, 1024, 512, 8, 8
HD = DIM // H
SCALE = HD ** -0.5
NCORES = 8
CHUNK = 128
NCHUNKS = N // CHUNK
BAND = 2  # +/- chunks around the diagonal where the local Gaussian matters

_PROGRAM_CACHE = {}


def _softplus64(x):
    x = np.asarray(x, np.float64)
    return np.log1p(np.exp(-np.abs(x))) + np.maximum(x, 0.0) + 1e-12


def _band(k):
    return max(0, k - BAND) * 128, min(NCHUNKS, k + BAND + 1) * 128


def _split512(c0, c1):
    """Split [c0,c1) at the 512-col PSUM bank boundary."""
    out = []
    if c0 < 512 and c1 > 512:
        out.append((c0, 512))
        out.append((512, c1))
    elif c1 > c0:
        out.append((c0, c1))
    return out


def _half_regions(h0, b0, b1):
    """Regions of [h0,h0+512) split at band [b0,b1): (c0, c1, inband)."""
    lo, hi = h0, h0 + 512
    bb0, bb1 = max(b0, lo), min(b1, hi)
    if bb0 >= bb1:
        return [(lo, hi, False)]
    out = []
    if lo < bb0:
        out.append((lo, bb0, False))
    out.append((bb0, bb1, True))
    if bb1 < hi:
        out.append((bb1, hi, False))
    return out


def _split_excess_waits(nc, max_waits=1):
    """CoreV3 walrus allows only one sync-wait command on some instruction
    encodings; move excess waits onto preceding same-engine NoOps."""
    import concourse.mybir as mybir
    import bass_rust

    n_split = 0
    for bb in nc.main_func.blocks:
        new_list = []
        changed = False
        for ins in bb.instructions:
            si = ins.sync_info
            waits = list(si.on_wait) if (si and si.on_wait) else []
            if len(waits) > max_waits:
                changed = True
                extra, keep = waits[:-max_waits], waits[-max_waits:]
                for i in range(0, len(extra), max_waits):
                    chunk = extra[i : i + max_waits]
                    n_split += 1
                    new_list.append(
                        mybir.InstNoOp(
                            name=f"{ins.name}-ws{i}",
                            engine=ins.engine,
                            ins=[],
                            outs=[],
                            sync_info=bass_rust.SyncInfo(
                                on_wait=chunk, on_update=[]
                            ),
                        )
                    )
                si.on_wait = keep
            new_list.append(ins)
        if changed:
            bb.instructions = new_list
    return n_split


def _build_program(biases_zero=True, repeat=1, debug=False):
    key = ("nc", biases_zero, repeat, debug)
    if key in _PROGRAM_CACHE:
        return _PROGRAM_CACHE[key]

    import concourse.bass as bass
    import concourse.mybir as mybir
    import concourse.tile as tile

    f32 = mybir.dt.float32
    f32r = mybir.dt.float32r
    f16 = mybir.dt.float16
    Alu = mybir.AluOpType
    Act = mybir.ActivationFunctionType

    nc = bass.Bass(trn_type="TRN2")

    # ---- per-core DRAM I/O (packed: one DMA per tensor) -------------------
    xt_d = nc.dram_tensor("xt", [128, 4 * N], f32r, kind="ExternalInput")
    identx_d = nc.dram_tensor("identx", [128, 129], f32r, kind="ExternalInput")
    wqk_d = nc.dram_tensor("wqk", [128, 4 * 256], f32r, kind="ExternalInput")
    wv_d = nc.dram_tensor("wv", [128, 4 * 128], f32r, kind="ExternalInput")
    wproj_d = nc.dram_tensor("wproj", [128, DIM], f32r, kind="ExternalInput")
    qb_d = nc.dram_tensor("qb", [2, HD, 1], f32, kind="ExternalInput")
    kb_d = nc.dram_tensor("kb", [2, HD, 1], f32, kind="ExternalInput")
    # 33 rows: C,S feats (16) | ones or -Ci (1) | sin-side combos (16);
    # heads side by side in the free dim
    kfx_d = nc.dram_tensor("kfx", [33, 2 * N], f32r, kind="ExternalInput")
    qpx_d = nc.dram_tensor("qpx", [33, 2 * N], f32r, kind="ExternalInput")
    qmx_d = nc.dram_tensor("qmx", [33, 2 * N], f32r, kind="ExternalInput")
    # per head: column-packed [qc | qcn | feat], each N wide (fp16)
    qcs_d = nc.dram_tensor("qcs", [16, 6 * N], f16, kind="ExternalInput")
    erlr_d = nc.dram_tensor("erlr", [3, 9 * N + 64], f32r, kind="ExternalInput")
    # fp16 pack: unit eye | ta_0-eye | ta_1-eye | 16 diag sin windows
    dw16_d = nc.dram_tensor("dw16", [128, 19 * 128], f16, kind="ExternalInput")
    yt_d = nc.dram_tensor("yt", [DIM, N], f16, kind="ExternalOutput")
    dbg_d = (
        nc.dram_tensor("dbg", [8 * 128, N], f32r, kind="ExternalOutput")
        if debug else None
    )
    dbg2_d = (
        nc.dram_tensor("dbg2", [2 * 128, N], f32r, kind="ExternalOutput")
        if debug else None
    )

    with tile.TileContext(nc) as tc:
      for _rep in range(repeat):
        with (
            tc.tile_pool(name="persist", bufs=1) as pers,
            tc.tile_pool(name="work", bufs=3) as work,
            tc.tile_pool(name="dmw", bufs=2) as dmw,
            tc.tile_pool(name="yg", bufs=2) as ygp,
        ):
            # ---- persistent SBUF tiles + input DMA, issued in the order the
            # prolog consumes them
            def pt(shape, tag, dt=f32):
                return pers.tile(shape, dt, tag=tag, name=tag)

            erlr_t = pt([3, 9 * N + 64], "erlr", f32r)
            nc.sync.dma_start(erlr_t[:], erlr_d[:])
            erl_t = erlr_t[:, 0:N]
            errr_t = erlr_t[:, N : 9 * N]
            e_t = [pt([128, N], f"e{k}", f16) for k in range(NCHUNKS)]
            xTall_t = pt([128, 4 * N], "xT", f32r)
            nc.sync.dma_start(xTall_t[:], xt_d[:])
            xT_t = [xTall_t[:, c * N : (c + 1) * N] for c in range(4)]
            wqkall_t = pt([128, 4 * 256], "wqk", f32r)
            nc.sync.dma_start(wqkall_t[:], wqk_d[:])
            wqk_t = [wqkall_t[:, kc * 256 : (kc + 1) * 256] for kc in range(4)]

            identx_t = pt([128, 129], "identx", f32r)
            nc.sync.dma_start(identx_t[:], identx_d[:])
            dw16_t = pt([128, 19 * 128], "dw16", f16)
            nc.sync.dma_start(dw16_t[:], dw16_d[:])
            ident16_t = dw16_t[:, 0:128]
            identa16_t = [dw16_t[:, 128:256], dw16_t[:, 256:384]]
            dwin_t = dw16_t[:, 384 : 19 * 128]
            qcsall_t = pt([16, 6 * N], "qcs", f16)
            nc.sync.dma_start(qcsall_t[:], qcs_d[:])
            qc_t, qcn_t, feat_t = [], [], []
            for h in range(2):
                b0 = h * 3 * N
                qc_t.append(qcsall_t[:, b0 : b0 + N])
                qcn_t.append(qcsall_t[:, b0 + N : b0 + 2 * N])
                feat_t.append(qcsall_t[:, b0 + 2 * N : b0 + 3 * N])

            kfall_t = pt([97, 2 * N], "kfall", f32r)
            nc.sync.dma_start(kfall_t[64:97, :], kfx_d[:])
            qapall_t = pt([97, 2 * N], "qapall", f32r)
            nc.sync.dma_start(qapall_t[64:97, :], qpx_d[:])
            qamall_t = pt([97, 2 * N], "qamall", f32r)
            nc.sync.dma_start(qamall_t[64:97, :], qmx_d[:])
            kf_t = [kfall_t[:, h * N : (h + 1) * N] for h in range(2)]
            qap_t = [qapall_t[:, h * N : (h + 1) * N] for h in range(2)]
            qam_t = [qamall_t[:, h * N : (h + 1) * N] for h in range(2)]
            qb_t, kb_t = [], []
            if not biases_zero:
                for h in range(2):
                    s = pt([HD, 1], f"qb{h}")
                    nc.sync.dma_start(s[:], qb_d[h])
                    qb_t.append(s)
                    s = pt([HD, 1], f"kb{h}")
                    nc.sync.dma_start(s[:], kb_d[h])
                    kb_t.append(s)
            wvall_t = pt([128, 4 * 128], "wvall", f32r)
            nc.sync.dma_start(wvall_t[:], wv_d[:])
            wv_t = [wvall_t[:, kc * 128 : (kc + 1) * 128] for kc in range(4)]
            wproj_t = pt([128, DIM], "wproj", f32r)
            nc.sync.dma_start(wproj_t[:], wproj_d[:])

            ones64_t = erlr_t[0:1, 9 * N : 9 * N + 64]
            onescol_t = identx_t[:, 128:129]
            vo_t = [[pt([128, 65], f"vo{h}_{t}", f32r) for t in range(8)] for h in range(2)]
            for h in range(2):
                for t in range(8):
                    nc.vector.tensor_copy(vo_t[h][t][:, 64:65], onescol_t)
            vT_t = pt([128, N], "vT", f32r)
            os_t = pt([128, N], "os", f32r)

            # ---- prolog: E, qk^T, v^T + transposes ------------------------
            with tc.tile_pool(name="ppro", bufs=2, space="PSUM") as ppro:
                # E(k) = exp(-d^2/ell^2): d^2 is rank-3 in the sorted coords;
                # band columns only.
                with tc.tile_pool(name="pe2", bufs=2, space="PSUM") as pep:
                    for k in range(NCHUNKS):
                        j0 = k * 128
                        b0, b1 = _band(k)
                        for c0, c1 in _split512(b0, b1):
                            pe2 = pep.tile([128, 512], f32, tag="pe2")
                            nc.tensor.matmul(
                                pe2[:, 0 : c1 - c0],
                                lhsT=erl_t[:, j0 : j0 + 128],
                                rhs=errr_t[:, k * N + c0 : k * N + c1],
                                start=True, stop=True, skip_group_check=True,
                            )
                            nc.scalar.activation(
                                e_t[k][:, c0:c1], pe2[:, 0 : c1 - c0], Act.Exp,
                            )

                # qk^T per head: rows 0:64 q, 64:128 k
                for h in range(2):
                    p = ppro.tile([128, N], f32, tag="ppro")
                    for kc in range(4):
                        for nh in range(2):
                            nc.tensor.matmul(
                                p[:, nh * 512 : (nh + 1) * 512],
                                lhsT=wqk_t[kc][:, h * 128 : (h + 1) * 128],
                                rhs=xT_t[kc][:, nh * 512 : (nh + 1) * 512],
                                start=(kc == 0),
                                stop=(kc == 3),
                            )
                    if biases_zero:
                        for nh in range(2):
                            cs_ = slice(nh * 512, (nh + 1) * 512)
                            nc.scalar.mul(
                                qap_t[h][0:64, cs_], p[0:64, cs_], SCALE
                            )
                            nc.vector.tensor_scalar_mul(
                                qam_t[h][0:64, cs_], p[0:64, cs_], SCALE
                            )
                            nc.vector.tensor_copy(
                                kf_t[h][0:64, cs_], p[64:128, cs_]
                            )
                    else:
                        nc.vector.tensor_scalar(
                            qap_t[h][0:64, :], p[0:64, :],
                            scalar1=qb_t[h][:], scalar2=SCALE,
                            op0=Alu.add, op1=Alu.mult,
                        )
                        nc.vector.tensor_scalar(
                            qam_t[h][0:64, :], p[0:64, :],
                            scalar1=qb_t[h][:], scalar2=SCALE,
                            op0=Alu.add, op1=Alu.mult,
                        )
                        nc.vector.tensor_scalar(
                            kf_t[h][0:64, :], p[64:128, :],
                            scalar1=kb_t[h][:], scalar2=None, op0=Alu.add,
                        )

                # v^T = wv^T x^T (both heads' 128 v-dims on partitions)
                with tc.tile_pool(name="pvt", bufs=2, space="PSUM") as pvt:
                    for nh in range(2):
                        pv = pvt.tile([128, 512], f32, tag="pvt")
                        for kc in range(4):
                            nc.tensor.matmul(
                                pv[:],
                                lhsT=wv_t[kc][:],
                                rhs=xT_t[kc][:, nh * 512 : (nh + 1) * 512],
                                start=(kc == 0),
                                stop=(kc == 3),
                            )
                        nc.scalar.copy(
                            vT_t[:, nh * 512 : (nh + 1) * 512], pv[:]
                        )
                    # flip back: vo[h][t][:, 0:64] = v rows for head h
                    for t in range(8):
                        ptr = pvt.tile([128, 512], f32r, tag="pvt")
                        nc.tensor.transpose(
                            ptr[:, 0:128],
                            vT_t[:, t * 128 : (t + 1) * 128],
                            identx_t[:, 0:128],
                        )
                        for h in range(2):
                            nc.vector.tensor_copy(
                                vo_t[h][t][:, 0:64],
                                ptr[:, h * 64 : (h + 1) * 64],
                            )

            # ---- main attention loop (attn@V pipelined one chunk back) ----
            with (
                tc.tile_pool(name="pp", bufs=2, space="PSUM") as ppp,
                tc.tile_pool(name="po", bufs=2, space="PSUM") as pop,
            ):
                def scores_chunk(h, k, p):
                    # per 512-col PSUM bank: exactly one full-width start=True
                    # matmul FIRST, then start=False accumulations, stop=True
                    # on the bank's last matmul (a mid-chain start on a bank
                    # clobbers earlier accumulation).
                    j0 = k * 128
                    ch = slice(j0, j0 + 128)
                    b0, b1 = _band(k)
                    for half in range(2):
                        h0c = half * 512
                        cols = slice(h0c, h0c + 512)
                        diag_half = h0c <= j0 < h0c + 512
                        eb0, eb1 = max(b0, h0c), min(b1, h0c + 512)
                        if diag_half:
                            nc.tensor.matmul(
                                p[:, cols], lhsT=kf_t[h][0:81, ch],
                                rhs=qap_t[h][0:81, cols],
                                start=True, stop=False, skip_group_check=True,
                            )
                            # sin corrections (fp16): off-chunk cols of this
                            # half via feature matmuls, in-chunk via the
                            # host-precomputed signed window
                            for c0, c1, neg in (
                                (h0c, j0, True),
                                (j0 + 128, h0c + 512, False),
                            ):
                                if c1 <= c0:
                                    continue
                                src = qcn_t[h] if neg else qc_t[h]
                                nc.tensor.matmul(
                                    p[:, c0:c1], lhsT=feat_t[h][:, ch],
                                    rhs=src[:, c0:c1],
                                    start=False, stop=False,
                                    skip_group_check=True,
                                )
                            nc.tensor.matmul(
                                p[:, ch], lhsT=ident16_t,
                                rhs=dwin_t[
                                    :, (h * 8 + k) * 128 : (h * 8 + k + 1) * 128
                                ],
                                start=False, stop=(eb1 <= eb0),
                                skip_group_check=True,
                            )
                        else:
                            src = qam_t[h] if j0 > h0c else qap_t[h]
                            nc.tensor.matmul(
                                p[:, cols], lhsT=kf_t[h][:, ch],
                                rhs=src[:, cols],
                                start=True, stop=(eb1 <= eb0),
                                skip_group_check=True,
                            )
                        # local Gaussian band add (fp16 scaled identity)
                        if eb1 > eb0:
                            nc.tensor.matmul(
                                p[:, eb0:eb1], lhsT=identa16_t[h],
                                rhs=e_t[k][:, eb0:eb1],
                                start=False, stop=True,
                                skip_group_check=True,
                            )
                    xb = work.tile([128, N], f32r, tag="xb")
                    nc.scalar.activation(xb[:], p[:], Act.Exp)
                    if debug and h == 0:
                        nc.sync.dma_start(
                            dbg_d[k * 128 : (k + 1) * 128, :], xb[:]
                        )
                    return xb

                def attnv_chunk(h, k, xb, o):
                    for nh in range(2):
                        nc.tensor.matmul(
                            o[0:65, nh * 512 : (nh + 1) * 512],
                            lhsT=vo_t[h][k][:],
                            rhs=xb[:, nh * 512 : (nh + 1) * 512],
                            start=(k == 0),
                            stop=(k == NCHUNKS - 1),
                            skip_group_check=True,
                        )

                for h in range(2):
                    o = pop.tile([128, N], f32, tag="po")
                    pending = None  # (k, xb) awaiting attn@V
                    for k in range(NCHUNKS):
                        p = ppp.tile([128, N], f32, tag="pp")
                        xb = scores_chunk(h, k, p)
                        if pending is not None:
                            attnv_chunk(h, pending[0], pending[1], o)
                        pending = (k, xb)
                    attnv_chunk(h, pending[0], pending[1], o)

                    # normalize: OS[h] = O[0:64] * (1/denominator)
                    rcp = dmw.tile([1, N], f32r, tag="rr")
                    with nc.allow_low_precision(reason="f32r is f32 bits"):
                        nc.vector.reciprocal(rcp[:], o[64:65, :])
                    rb = ppp.tile([64, N], f32, tag="pp")
                    for nh in range(2):
                        nc.tensor.matmul(
                            rb[:, nh * 512 : (nh + 1) * 512],
                            lhsT=ones64_t[:],
                            rhs=rcp[:, nh * 512 : (nh + 1) * 512],
                            start=True, stop=True, skip_group_check=True,
                        )
                    rbs = dmw.tile([64, N], f32r, tag="rbs")
                    nc.scalar.copy(rbs[:], rb[:])
                    nc.vector.tensor_tensor(
                        os_t[h * 64 : (h + 1) * 64, :],
                        o[0:64, :],
                        rbs[:],
                        op=Alu.mult,
                    )

                if debug:
                    nc.sync.dma_start(dbg2_d[0:128, :], vT_t[:])
                    nc.sync.dma_start(dbg2_d[128:256, :], os_t[:])

                # ---- projection partial ----------------------------------
                for g in range(4):
                    p = ppp.tile([128, N], f32, tag="pp")
                    for nh in range(2):
                        nc.tensor.matmul(
                            p[:, nh * 512 : (nh + 1) * 512],
                            lhsT=wproj_t[:, g * 128 : (g + 1) * 128],
                            rhs=os_t[:, nh * 512 : (nh + 1) * 512],
                            start=True, stop=True, skip_group_check=True,
                        )
                    yg = ygp.tile([128, N], f16, tag="yg")
                    nc.scalar.copy(yg[:, 0:512], p[:, 0:512])
                    nc.vector.tensor_copy(yg[:, 512:N], p[:, 512:N])
                    nc.sync.dma_start(yt_d[g * 128 : (g + 1) * 128, :], yg[:])

    _split_excess_waits(nc)
    _PROGRAM_CACHE[key] = nc
    return nc


def _prepare_in_maps(
    x_tokens, coords, qkv_w, qkv_b, proj_w, omega_raw, a, c,
    alpha_raw, ell_raw, bias_scale_raw,
):
    """Host-side preprocessing. Returns (in_maps, perms)."""
    x64 = np.asarray(x_tokens, np.float64)
    co64 = np.asarray(coords, np.float64)
    w64 = np.asarray(qkv_w, np.float64)
    wb64 = np.asarray(qkv_b, np.float64)

    alpha = _softplus64(alpha_raw)            # (H,)
    ell = _softplus64(ell_raw)                # (H,)
    om = _softplus64(omega_raw)               # (H, F)
    t = np.tanh(np.asarray(bias_scale_raw, np.float64))  # (H,)
    a2 = t[:, None] * np.asarray(a, np.float64)          # (H, F)
    c2 = t[:, None] * np.asarray(c, np.float64)
    ta = t * alpha                                        # (H,)

    assert np.allclose(ell, ell[0]), "per-head ell not supported"

    io, jo = np.meshgrid(np.arange(128), np.arange(128), indexing="ij")
    tri = np.sign(jo - io).astype(np.float64)  # TRI[p, c] = sign(c - p)

    perms, in_maps = [], []
    for b in range(B):
        perm = np.argsort(co64[b], kind="stable")
        perms.append(perm)
        cs = co64[b][perm]                      # sorted coords
        xs = x64[b][perm]                       # (N, DIM)
        l2 = ell[0] ** 2
        # sanity: outside the +/-BAND chunk band the local term must vanish
        for k in range(NCHUNKS):
            lo_edge = cs[max(0, (k - BAND)) * 128 - 1] if k > BAND else None
            hi_edge = (
                cs[(k + BAND + 1) * 128] if k + BAND + 1 < NCHUNKS else None
            )
            cmin, cmax = cs[k * 128], cs[(k + 1) * 128 - 1]
            for edge, ref in ((lo_edge, cmin), (hi_edge, cmax)):
                if edge is not None:
                    d2 = (edge - ref) ** 2
                    assert np.exp(-d2 / l2) * np.abs(ta).max() < 5e-3, (
                        "BAND too small for these coords"
                    )
        # per-key-chunk centering keeps the rank-3 d^2 factors small where
        # E = exp(-d^2/l^2) is non-negligible (f32r has ~12 mantissa bits)
        erl = np.empty((3, N))
        errr = np.empty((NCHUNKS, 3, N))
        for kk in range(NCHUNKS):
            mu = cs[kk * 128 : (kk + 1) * 128].mean()
            u = cs[kk * 128 : (kk + 1) * 128] - mu
            erl[:, kk * 128 : (kk + 1) * 128] = np.stack(
                [u**2, u, np.ones(128)]
            )
            w = cs - mu
            errr[kk] = np.stack([-np.ones(N) / l2, 2 * w / l2, -(w**2) / l2])
        erl = erl.astype(np.float32)
        # (NCHUNKS,3,N) -> (3, NCHUNKS*N) with chunk-k block at cols [k*N,(k+1)*N)
        errr = np.concatenate(list(errr), axis=1).astype(np.float32)

        # all-head q/k in f64 for the per-query shift bound
        qk = xs @ w64[:, : 2 * DIM] + wb64[: 2 * DIM]
        for pair in range(4):
            heads = (2 * pair, 2 * pair + 1)
            wqk_cols, wv_cols = [], []
            qb_rows, kb_rows = [], []
            kext, qextp, qextm, qcs = [], [], [], []
            identa, dwin = [], []
            for h in heads:
                sl_q = slice(h * HD, (h + 1) * HD)
                sl_k = slice(DIM + h * HD, DIM + (h + 1) * HD)
                sl_v = slice(2 * DIM + h * HD, 2 * DIM + (h + 1) * HD)
                wqk_cols.append(np.asarray(qkv_w)[:, sl_q])
                wqk_cols.append(np.asarray(qkv_w)[:, sl_k])
                wv_cols.append(np.asarray(qkv_w)[:, sl_v])
                qb_rows.append(np.asarray(qkv_b)[sl_q])
                kb_rows.append(np.asarray(qkv_b)[sl_k])

                C = np.cos(om[h][:, None] * cs[None, :])   # (F, N)
                S = np.sin(om[h][:, None] * cs[None, :])
                kext.append(np.concatenate([C, S, np.ones((1, N)), C, S], axis=0))
                qn = np.linalg.norm(qk[:, sl_q], axis=1)   # (N,)
                kmax = np.linalg.norm(qk[:, sl_k], axis=1).max()
                bb = abs(ta[h]) + np.abs(a2[h]).sum() + np.abs(c2[h]).sum()
                ci = SCALE * qn * kmax + bb + 1.0          # (N,)
                qa_rows = np.concatenate(
                    [a2[h][:, None] * C, a2[h][:, None] * S, -ci[None, :]],
                    axis=0,
                )  # (17, N)
                qc_rows = np.concatenate(
                    [c2[h][:, None] * S, -c2[h][:, None] * C], axis=0
                )  # (16, N)
                qextp.append(np.concatenate([qa_rows, qc_rows], axis=0))
                qextm.append(np.concatenate([qa_rows, -qc_rows], axis=0))
                qcs.append(
                    np.concatenate(
                        [qc_rows, -qc_rows, np.concatenate([C, S], axis=0)],
                        axis=1,
                    )
                )
                identa.append(np.eye(128) * ta[h])
                # signed sin-part on each diagonal 128x128 window:
                # dwin[k][j, i] = tri[j, i] * sum_r featCS[r, j0+j] qc[r, j0+i]
                featcs = np.concatenate([C, S], axis=0)      # (16, N)
                wins = []
                for k in range(NCHUNKS):
                    j0 = k * 128
                    blk = featcs[:, j0 : j0 + 128].T @ qc_rows[:, j0 : j0 + 128]
                    wins.append(blk * tri)
                dwin.append(np.stack(wins))

            def rowpack(arr):
                rr = arr.shape[0] // 128
                return np.ascontiguousarray(
                    np.concatenate(
                        [arr[r * 128 : (r + 1) * 128] for r in range(rr)], axis=1
                    ),
                    dtype=np.float32,
                )

            in_maps.append(
                {
                    "xt": rowpack(xs.T),
                    "identx": np.concatenate(
                        [np.eye(128), np.ones((128, 1))], axis=1
                    ).astype(np.float32),
                    "wqk": rowpack(np.concatenate(wqk_cols, axis=1)),
                    "wv": rowpack(np.concatenate(wv_cols, axis=1)),
                    "wproj": np.ascontiguousarray(
                        np.asarray(proj_w)[
                            heads[0] * HD : (heads[1] + 1) * HD, :
                        ],
                        dtype=np.float32,
                    ),
                    "qb": np.stack(qb_rows).astype(np.float32)[:, :, None],
                    "kb": np.stack(kb_rows).astype(np.float32)[:, :, None],
                    "kfx": np.concatenate(kext, axis=1).astype(np.float32),
                    "qpx": np.concatenate(qextp, axis=1).astype(np.float32),
                    "qmx": np.concatenate(qextm, axis=1).astype(np.float32),
                    "qcs": np.concatenate(qcs, axis=1).astype(np.float16),
                    "erlr": np.concatenate(
                        [erl, errr, np.ones((3, 64))], axis=1
                    ).astype(np.float32),
                    "dw16": np.concatenate(
                        [np.eye(128)] + identa + [w for hw in dwin for w in hw],
                        axis=1,
                    ).astype(np.float16),
                }
            )
    return in_maps, perms


def kernel(
    x_tokens, coords, qkv_w, qkv_b, proj_w, proj_b,
    omega_raw, a, c, alpha_raw, ell_raw, bias_scale_raw,
):
    from concourse.bass_utils import run_bass_kernel_spmd

    biases_zero = not np.any(np.asarray(qkv_b))
    nc = _build_program(biases_zero=biases_zero)
    in_maps, perms = _prepare_in_maps(
        x_tokens, coords, qkv_w, qkv_b, proj_w, omega_raw, a, c,
        alpha_raw, ell_raw, bias_scale_raw,
    )
    res = run_bass_kernel_spmd(nc, in_maps, core_ids=list(range(NCORES)))

    # v-bias contributes a constant row (attention weights sum to 1)
    vb = np.asarray(qkv_b, np.float64)[2 * DIM :]
    const_row = vb @ np.asarray(proj_w, np.float64) + np.asarray(
        proj_b, np.float64
    )

    out = np.empty((B, N, DIM), np.float32)
    for b in range(B):
        acc = np.zeros((N, DIM), np.float64)
        for pair in range(4):
            acc += res.results[4 * b + pair]["yt"].T.astype(np.float64)
        acc += const_row[None, :]
        y = np.empty((N, DIM), np.float64)
        y[perms[b]] = acc
        out[b] = y.astype(np.float32)
    return out
